# revision 43
# baseline (speedup 1.0000x reference)
"""Causal self-attention (B=4, T=2048, C=1024, H=16) on 8 Trainium2 cores.

Sharding: data-parallel over batch (4 groups) x tensor-parallel over heads
(2-way). Core c = 2*b + t handles batch b, heads [t*8, t*8+8).

v3 design notes (flat ACT:PE ratio schedule):
  v2 ran qc-major phases; its final phase (qc=3 attention) had per-window
  scalar-engine exp load ~= PE load, so the PE starved behind exp, the DVFS
  controller dropped the PE clock to ~1.2 GHz for the last ~90us, and every
  tail matmul ran ~2x slow.  v3 keeps v2's math and per-unit structure but
  re-orders the whole kernel as two blocks with a uniform exp:matmul mix:

  - block A: per head, pair (0,h) then (1,h)   (10 units/head)
  - block B: per head, pair (2,h) then (3,h)   (18 units/head)
  Projection m-tiles are spread just-in-time as PE fillers so every window
  keeps PE work ~1.2x the exp work: vproj(0..1)+qkproj(0..1) inside A,
  vproj(2..3)+qkproj(2..3)+oproj(0..1) inside B, oproj(2) in the (3,h7)
  window, oproj(3) staged as partials into the drain.

  Engine re-balance: the PSUM->SBUF moves that v2 ran on DVE/ACT (qk bias
  add, v bias add, early oproj epilogues) go to the mostly-idle Pool
  (gpsimd) engine; ACT does exp only; DVE keeps tri-mask, l-copy/recip and
  the softmax normalize; output DMA issues from the SP (sync) queue instead
  of Pool's SWDGE.  S->AV lag raised to 4 units for more exp-latency slack.

Per-core math (all matmuls fp16 in / fp32 psum accumulate), as in v2:
  qkv in transposed layout q^T/k^T[feat,T]; V in [T,feat] with a ones column
  so AV also accumulates the softmax denominator; S^T = K^T.T @ Q^T per
  (head, k-block, 512-col q-chunk), causal blocks only; exp on ACT, fused
  across the unit's 2 PSUM banks; multiplicative f16 triangle mask after exp
  on the diagonal blocks; out^T row-parallel = W_proj_half.T @ y^T (+ b_proj
  on the t=0 core); host sums TP partials.
"""

import os
import sys

import numpy as np

from concourse import mybir, tile, bacc
from concourse import bass_utils
from concourse.bass_utils import run_bass_kernel_spmd


def _ensure_trace_support():
    """Make trace=True / BASS_TRACE runs survive on images whose antenv lacks
    axon_hooks and where artifact upload has no credentials. Both shims are
    no-ops on the untraced path."""
    try:
        import antenv.axon_hooks  # noqa: F401
    except ImportError:
        import contextlib
        import ctypes
        import types

        mod = types.ModuleType("antenv.axon_hooks")
        state = {"hook": None, "tried": False}

        def set_axon_ntff_profile_hook(hook):
            state["hook"] = hook

        def _via_ctypes(so_path):
            lib = ctypes.CDLL(so_path)
            if not hasattr(lib, "axon_start_nrt_profile"):
                return None
            lib.axon_start_nrt_profile.argtypes = [
                ctypes.POINTER(ctypes.c_int64),
                ctypes.c_size_t,
            ]
            lib.axon_start_nrt_profile.restype = ctypes.c_int64
            lib.axon_stop_nrt_profile.argtypes = [ctypes.c_char_p]
            lib.axon_stop_nrt_profile.restype = ctypes.c_int64

            @contextlib.contextmanager
            def _hook(output_dir, device_ids):
                import jax

                jax.devices()
                if device_ids:
                    ids = (ctypes.c_int64 * len(device_ids))(*device_ids)
                    rc = lib.axon_start_nrt_profile(ids, len(device_ids))
                else:
                    rc = lib.axon_start_nrt_profile(None, 0)
                if rc != 0:
                    raise RuntimeError(f"axon_start_nrt_profile rc={rc}")
                try:
                    yield
                finally:
                    lib.axon_stop_nrt_profile(str(output_dir).encode())

            return _hook

        def get_axon_ntff_profile_hook():
            if state["hook"] is None and not state["tried"]:
                state["tried"] = True
                so = os.environ.get("AXON_PJRT_SO", "/opt/axon/libaxon_pjrt.so")
                if os.path.exists(so):
                    try:
                        state["hook"] = _via_ctypes(so)
                    except OSError:
                        pass
            return state["hook"]

        mod.set_axon_ntff_profile_hook = set_axon_ntff_profile_hook
        mod.get_axon_ntff_profile_hook = get_axon_ntff_profile_hook
        sys.modules["antenv.axon_hooks"] = mod

    orig_upload = bass_utils.upload_artifacts
    if not getattr(orig_upload, "_safe_wrapped", False):
        def _safe_upload(tmpdir):
            try:
                return orig_upload(tmpdir)
            except Exception:
                return "local://" + str(tmpdir)

        _safe_upload._safe_wrapped = True
        bass_utils.upload_artifacts = _safe_upload


_ensure_trace_support()

F16 = mybir.dt.float16
F32 = mybir.dt.float32
EXPF = mybir.ActivationFunctionType.Exp
IDF = mybir.ActivationFunctionType.Identity

B, T, C, H, D = 4, 2048, 1024, 16, 64
HPC = 8          # heads per core
QC = 512         # q-chunk width
NT = T // 128    # 16 T-tiles of 128
NQC = T // QC    # 4 q-chunks
NKC = C // 128   # 8 contraction tiles for the input projections
NKP = (HPC * D) // 128  # 4 contraction tiles for the output projection
LAG = int(os.environ.get("K_LAG", "4"))  # units between an S group and its AV

_CACHE = {}


def _build():
    nc = bacc.Bacc("TRN2", target_bir_lowering=False, debug=False)

    xT = nc.dram_tensor("xT", [C, T], F16, kind="ExternalInput")
    wqk = nc.dram_tensor("wqk", [C, HPC * 128], F16, kind="ExternalInput")
    bqk = nc.dram_tensor("bqk", [HPC * 128], F32, kind="ExternalInput")
    wv = nc.dram_tensor("wv", [C, HPC * D], F16, kind="ExternalInput")
    bv = nc.dram_tensor("bv", [HPC * D], F32, kind="ExternalInput")
    wp = nc.dram_tensor("wp", [HPC * D, C], F16, kind="ExternalInput")
    bp = nc.dram_tensor("bp", [C], F32, kind="ExternalInput")
    tri = nc.dram_tensor("tri", [128, 128], F16, kind="ExternalInput")
    outT = nc.dram_tensor("outT", [C, T], F32, kind="ExternalOutput")

    with tile.TileContext(nc) as tc:
        with (
            tc.tile_pool(name="wts", bufs=1) as wpool,
            tc.tile_pool(name="qk", bufs=1) as qkpool,
            tc.tile_pool(name="vy", bufs=1) as vypool,
            tc.tile_pool(name="xc", bufs=4) as xpool,
            tc.tile_pool(name="pt", bufs=LAG + 2) as ptpool,
            tc.tile_pool(name="st", bufs=3) as stpool,
            tc.tile_pool(name="sm", bufs=3) as smallpool,
            tc.tile_pool(name="sg", bufs=2, space="PSUM") as spool,
            tc.tile_pool(name="ob", bufs=2, space="PSUM") as opool,
            tc.tile_pool(name="pp", bufs=2, space="PSUM") as ppool,
        ):
            # ---- persistent activation tiles ----
            # head-pair packing: tile 2g = q^T of heads 2g (rows 0-63) and
            # 2g+1 (rows 64-127); tile 2g+1 = k^T of the same pair.  Head h
            # reads q and k at the same base partition 64*(h%2) (a matmul
            # requirement), and each qkproj PSUM drain is one full-width op.
            qk_t = [qkpool.tile([128, T], F16, tag=f"qk{m}", name=f"qk{m}") for m in range(HPC)]
            v_sb = vypool.tile([128, NT, HPC, D + 1], F16, tag="v")
            nc.vector.memset(v_sb[:, :, :, D : D + 1], 1.0)
            y_t = [vypool.tile([128, T], F16, tag=f"y{g}", name=f"y{g}") for g in range(NKP)]

            # ---- startup DMAs in consumption order: first V m-tile needs
            # xc0 + wv; first qk m-tile needs wqk half + bqk; tri is needed by
            # the very first diag unit so it goes out early ----
            xcs = {}
            def dma_xc(qc):
                xc = xpool.tile([128, NKC, QC], F16, tag="xc")
                src = xT.ap()[:, qc * QC : (qc + 1) * QC].rearrange("(a p) n -> p a n", p=128)
                nc.sync.dma_start(xc[:], src)
                xcs[qc] = xc

            # startup order: xc0 (kk-pair chunks, streamed into qkproj's
            # accumulation), bqk, wqk head-pair-0 quarter -- that 1.5MB is all
            # qkproj(0,0)/(0,1) need, so S units start ~4.5us in; wv/bvb/tri
            # follow for the vproj prologue, then the rest of the weights
            xc0 = xpool.tile([128, NKC, QC], F16, tag="xc")
            xc0_src = xT.ap()[:, 0:QC].rearrange("(a p) n -> p a n", p=128)
            wv_sb = wpool.tile([128, NKC, HPC * D], F16, tag="wv")
            wv_src = wv.ap().rearrange("(a p) m -> p a m", p=128)
            wqk_sb = wpool.tile([128, NKC, HPC * 128], F16, tag="wqk")
            wqk_src = wqk.ap().rearrange("(a p) m -> p a m", p=128)
            for kk2 in range(0, NKC, 2):
                nc.sync.dma_start(xc0[:, kk2 : kk2 + 2, :], xc0_src[:, kk2 : kk2 + 2, :])
            xcs[0] = xc0
            bqk_sb = wpool.tile([128, HPC], F32, tag="bqk")
            nc.sync.dma_start(bqk_sb[:], bqk.ap().rearrange("(m p) -> p m", p=128))
            nc.sync.dma_start(wqk_sb[:, :, 0:256], wqk_src[:, :, 0:256])
            for kk2 in range(0, NKC, 2):
                nc.sync.dma_start(wv_sb[:, kk2 : kk2 + 2, :], wv_src[:, kk2 : kk2 + 2, :])
            bvb = wpool.tile([128, HPC * D], F32, tag="bvb")
            nc.sync.dma_start(
                bvb[:],
                bv.ap().rearrange("(o n) -> o n", o=1).partition_broadcast(128),
            )
            # two copies of the triangle side by side so one DVE multiply
            # masks both slots of a diag pair's pt tile
            trisb = wpool.tile([128, 2 * 128], F16, tag="tri")
            nc.sync.dma_start(trisb[:, 0:128], tri.ap())
            nc.sync.dma_start(trisb[:, 128:256], tri.ap())
            for quarter in range(1, 4):
                s = slice(quarter * 256, (quarter + 1) * 256)
                nc.sync.dma_start(wqk_sb[:, :, s], wqk_src[:, :, s])
            dma_xc(1)
            wp_sb = wpool.tile([128, NKP, C], F16, tag="wp")
            nc.sync.dma_start(wp_sb[:], wp.ap().rearrange("(a p) m -> p a m", p=128))
            bp_sb = wpool.tile([128, C // 128], F32, tag="bp")
            nc.sync.dma_start(bp_sb[:], bp.ap().rearrange("(m p) -> p m", p=128))
            dma_xc(2)
            dma_xc(3)

            # ---- projection m-tiles (used as attention fillers) ----
            def vproj_mtile(qc, tt):
                ps = ppool.tile([128, QC], F32, tag="pp", name="psv")
                for kk in range(NKC):
                    nc.tensor.matmul(
                        ps[:],
                        xcs[qc][:, kk, tt * 128 : (tt + 1) * 128],
                        wv_sb[:, kk, :],
                        start=(kk == 0),
                        stop=(kk == NKC - 1),
                    )
                nc.vector.tensor_add(
                    v_sb[:, qc * 4 + tt, :, 0:D],
                    ps[:].rearrange("p (h d) -> p h d", d=D),
                    bvb[:].rearrange("p (h d) -> p h d", d=D),
                )

            def qkproj_mtile(qc, m):
                q0 = qc * QC
                ps = ppool.tile([128, QC], F32, tag="pp", name="psqk")
                for kk in range(NKC):
                    nc.tensor.matmul(
                        ps[:],
                        wqk_sb[:, kk, m * 128 : (m + 1) * 128],
                        xcs[qc][:, kk, :],
                        start=(kk == 0),
                        stop=(kk == NKC - 1),
                    )
                nc.vector.tensor_scalar_add(
                    qk_t[m][:, q0 : q0 + QC], ps[:], bqk_sb[:, m : m + 1]
                )

            def oproj_epilogue(qc, m, pps):
                q0 = qc * QC
                st = stpool.tile([128, QC], F32, tag="st", name="st")
                # chunk-3 epilogues drain after the last exp: split them
                # across ACT and DVE so the final PSUM drain runs in parallel
                if qc == 3 and m % 2 == 0:
                    nc.scalar.activation(st[:], pps[:], IDF, bias=bp_sb[:, m : m + 1])
                else:
                    nc.vector.tensor_scalar_add(st[:], pps[:], bp_sb[:, m : m + 1])
                nc.gpsimd.dma_start(
                    outT.ap()[m * 128 : (m + 1) * 128, q0 : q0 + QC], st[:]
                )
                return st

            def oproj_mtile_mm(qc, m):
                q0 = qc * QC
                pps = ppool.tile([128, QC], F32, tag="pp", name="pso")
                for kk in range(NKP):
                    nc.tensor.matmul(
                        pps[:],
                        wp_sb[:, kk, m * 128 : (m + 1) * 128],
                        y_t[kk][:, q0 : q0 + QC],
                        start=(kk == 0),
                        stop=(kk == NKP - 1),
                    )
                return pps

            def oproj_mtile(qc, m):
                oproj_epilogue(qc, m, oproj_mtile_mm(qc, m))

            # chunk-3 output projection, split so only the kk=3 matmuls (which
            # need the very last normalizes) remain after the unit stream
            # drains. m 0-3 borrow freed S-group banks; m 4,5 use the proj
            # pool (allocated after all other pp users, released by finals).
            o3ps = {}

            def oproj3_partial(m):
                q0 = 3 * QC
                if m in (0, 1):
                    if "A" not in o3ps:
                        o3ps["A"] = spool.tile([128, 2, QC], F32, tag="sg", name="o3A")
                    pps = o3ps["A"][:, m % 2, :]
                elif m in (2, 3):
                    if "B" not in o3ps:
                        o3ps["B"] = spool.tile([128, 2, QC], F32, tag="sg", name="o3B")
                    pps = o3ps["B"][:, m % 2, :]
                else:
                    pps = ppool.tile([128, QC], F32, tag="pp", name="pso3")[:]
                o3ps[m] = pps
                for kk in range(3):
                    nc.tensor.matmul(
                        pps,
                        wp_sb[:, kk, m * 128 : (m + 1) * 128],
                        y_t[kk][:, q0 : q0 + QC],
                        start=(kk == 0),
                        stop=False,
                    )

            def oproj3_final_mm(m):
                q0 = 3 * QC
                nc.tensor.matmul(
                    o3ps[m],
                    wp_sb[:, 3, m * 128 : (m + 1) * 128],
                    y_t[3][:, q0 : q0 + QC],
                    start=False,
                    stop=True,
                )

            # ---- attention unit stream ----
            # unit = ("full", qc, h, g) -> k-blocks 2g, 2g+1 (S pair + fused exp)
            #      | ("diag", qc, h, o) -> k-block 4qc+o, partial width + tri mask
            # Block A: per head, pair (0,h) then (1,h).  Block B: per head,
            # pair (2,h) then (3,h).  This keeps the exp:matmul ratio flat so
            # the scalar engine never backs the PE up (v2's tail problem).
            units = []
            for h in range(HPC):
                units += [("diag", 0, h, o) for o in range(4)]
                units += [("full", 1, h, g) for g in range(2)]
                units += [("diag", 1, h, o) for o in range(4)]
            baseB = len(units)  # 80
            for h in range(HPC):
                units += [("full", 2, h, g) for g in range(4)]
                units += [("diag", 2, h, o) for o in range(4)]
                units += [("full", 3, h, g) for g in range(6)]
                units += [("diag", 3, h, o) for o in range(4)]
            nunits = len(units)  # 224

            # fillers, just-in-time: position p emits after unit p's S (and
            # unit p-LAG's AV), so a filler feeding unit j sits at p <= j-1
            fill_at = {}

            def fill(pos, fn):
                fill_at.setdefault(pos, []).append(fn)

            # block A: vproj(0) under head 0's first units, vproj(1) under its
            # (1,h0) units; each head-pair g's four qkproj m-tiles (Q/K for
            # chunks 0 and 1) spread over the two preceding head windows
            fill(0, lambda: vproj_mtile(0, 1))
            fill(1, lambda: vproj_mtile(0, 2))
            fill(1, lambda: qkproj_mtile(1, 0))
            fill(2, lambda: vproj_mtile(0, 3))
            fill(2, lambda: qkproj_mtile(1, 1))
            for tt in range(4):
                fill(5 + tt, lambda tt=tt: vproj_mtile(1, tt))
            for g in range(1, 4):
                fill(20 * g - 15, lambda m=2 * g: qkproj_mtile(0, m))
                fill(20 * g - 9, lambda m=2 * g + 1: qkproj_mtile(0, m))
                fill(20 * g - 3, lambda m=2 * g: qkproj_mtile(1, m))
                fill(20 * g + 1, lambda m=2 * g + 1: qkproj_mtile(1, m))
            fill(63, lambda: qkproj_mtile(2, 0))
            fill(68, lambda: qkproj_mtile(2, 1))
            # block B: vproj(2)/vproj(3) under head 0's windows, qkproj(3,*)
            # early in each pair, qkproj(2,*) for the next pair late in it,
            # oproj(0)/oproj(1) one m-tile per head window
            for tt in range(4):
                fill(baseB + tt, lambda tt=tt: vproj_mtile(2, tt))
                fill(baseB + 9 + tt, lambda tt=tt: vproj_mtile(3, tt))
            for g in range(4):
                b0, b1 = baseB + 36 * g, baseB + 36 * g + 18
                fill(b0 + 3, lambda m=2 * g: qkproj_mtile(3, m))
                fill(b1 + 1, lambda m=2 * g + 1: qkproj_mtile(3, m))
                if g < 3:
                    fill(b0 + 15, lambda m=2 * g + 2: qkproj_mtile(2, m))
                    fill(b1 + 15, lambda m=2 * g + 3: qkproj_mtile(2, m))
            for h in range(HPC):
                base = baseB + 18 * h
                fill(base + 4, lambda h=h: oproj_mtile(0, h))
                fill(base + 13 if h < 7 else baseB + 18 * 6 + 15,
                     lambda h=h: oproj_mtile(1, h))
            # oproj(2) lands in the (3,h7) window right after (2,h7) is
            # normalized -- exactly where the unit stream runs out of cheap
            # filler; oproj(3) partials go into the drain (the spool-backed
            # ones must follow the last S unit or the pool ring deadlocks)
            for m in range(HPC):
                fill(nunits - 5 + m // 2, lambda m=m: oproj_mtile(2, m))
            fill(nunits + 0, lambda: oproj3_partial(4))
            fill(nunits + 1, lambda: oproj3_partial(5))
            fill(nunits + 2, lambda: oproj3_partial(0))
            fill(nunits + 2, lambda: oproj3_partial(1))
            fill(nunits + 3, lambda: oproj3_partial(2))
            fill(nunits + 3, lambda: oproj3_partial(3))

            o_tiles = {}   # (qc, h) -> O psum tile
            dsg = {}       # (qc, h) -> sgrp tile shared by diag pairs
            pts = {}       # unit index -> pt tile

            def emit_S(i):
                kind, qc, h, a = units[i]
                q0 = qc * QC
                if kind == "full":
                    sg = spool.tile([128, 2, QC], F32, tag="sg", name="sg")
                    qt, kt = qk_t[2 * (h // 2)], qk_t[2 * (h // 2) + 1]
                    b0 = 64 * (h % 2)
                    for b in (0, 1):
                        j = 2 * a + b
                        nc.tensor.matmul(
                            sg[:, b, :],
                            kt[b0 : b0 + 64, j * 128 : (j + 1) * 128],
                            qt[b0 : b0 + 64, q0 : q0 + QC],
                            start=True,
                            stop=True,
                        )
                    pt = ptpool.tile([128, 2, QC], F16, tag="pt2", name="pt2")
                    nc.scalar.activation(
                        pt[:].rearrange("p a n -> p (a n)"),
                        sg[:].rearrange("p a n -> p (a n)"),
                        EXPF,
                        scale=0.125,
                    )
                else:
                    o = a
                    if o % 2 == 0:
                        sg = spool.tile([128, 2, QC], F32, tag="sg", name="sgd")
                        dsg[(qc, h)] = sg
                    else:
                        sg = dsg[(qc, h)]
                    j = 4 * qc + o
                    c0 = 128 * o
                    w = QC - c0
                    qt, kt = qk_t[2 * (h // 2)], qk_t[2 * (h // 2) + 1]
                    b0 = 64 * (h % 2)
                    nc.tensor.matmul(
                        sg[:, o % 2, 0:w],
                        kt[b0 : b0 + 64, j * 128 : (j + 1) * 128],
                        qt[b0 : b0 + 64, q0 + c0 : q0 + QC],
                        start=True,
                        stop=True,
                    )
                    if o % 2 == 0:
                        # o pairs (0,1) / (2,3) share one pt tile, one tri
                        # multiply, and (for 0/1) one fused exp -- the shared
                        # ops let the second AV's PE wait be elided
                        pts[i] = None
                        return
                    pt = ptpool.tile([128, 2, QC], F16, tag="pt2", name="ptd")
                    if o == 1:
                        # fused exp over both slots (slot-1 cols 384: hold
                        # stale finite S values whose exps are never read)
                        nc.scalar.activation(
                            pt[:].rearrange("p a n -> p (a n)"),
                            sg[:].rearrange("p a n -> p (a n)"),
                            EXPF,
                            scale=0.125,
                        )
                    else:
                        nc.scalar.activation(
                            pt[:, 0, 0:256], sg[:, 0, 0:256], EXPF, scale=0.125
                        )
                        nc.scalar.activation(
                            pt[:, 1, 0:128], sg[:, 1, 0:128], EXPF, scale=0.125
                        )
                    nc.vector.tensor_mul(
                        pt[:, :, 0:128],
                        pt[:, :, 0:128],
                        trisb[:].rearrange("p (a n) -> p a n", a=2),
                    )
                    pts[i - 1] = (pt, 0)
                    pts[i] = (pt, 1)
                    return
                pts[i] = pt

            def emit_AV(i):
                kind, qc, h, a = units[i]
                n_k = 4 * qc + 4
                if (qc, h) not in o_tiles:
                    o_tiles[(qc, h)] = opool.tile([128, QC], F32, tag="ob", name="ob")
                ot = o_tiles[(qc, h)]
                pt = pts.pop(i)
                if kind == "full":
                    for b in (0, 1):
                        j = 2 * a + b
                        nc.tensor.matmul(
                            ot[0:65, :],
                            v_sb[:, j, h, :],
                            pt[:, b, :],
                            start=(j == 0),
                            stop=(j == n_k - 1),
                        )
                else:
                    o = a
                    j = 4 * qc + o
                    c0 = 128 * o
                    src = pt[0][:, pt[1], 0 : QC - c0] if isinstance(pt, tuple) else pt[:, 0 : QC - c0]
                    nc.tensor.matmul(
                        ot[0:65, c0:QC],
                        v_sb[:, j, h, :],
                        src,
                        start=(j == 0),
                        stop=(j == n_k - 1),
                    )

            def emit_recip(qc, h):
                # copy l out of PSUM first: the approx-fast custom op's
                # BITWISE_NOT seed reads garbage through the PSUM port
                lrow = smallpool.tile([1, QC], F32, tag="lr", name="lr")
                nc.vector.tensor_copy(lrow[:], o_tiles[(qc, h)][64:65, :])
                r = smallpool.tile([1, QC], F32, tag="r", name="r")
                nc.vector.reciprocal_approx_fast(r[:], lrow[:])
                return r

            def emit_norm(qc, h, r):
                rb = smallpool.tile([64, QC], F32, tag="rb", name="rb")
                nc.gpsimd.partition_broadcast(rb[:], r[:])
                r0 = (h % 2) * 64
                nc.vector.tensor_mul(
                    y_t[h // 2][r0 : r0 + 64, qc * QC : (qc + 1) * QC],
                    o_tiles.pop((qc, h))[0:64, :],
                    rb[:],
                )

            # ---- prologue + pipeline loop ----
            qkproj_mtile(0, 0)
            qkproj_mtile(0, 1)
            vproj_mtile(0, 0)

            recips = {}
            for i in range(nunits + LAG + 2):
                if i < nunits:
                    emit_S(i)
                iav = i - LAG
                if 0 <= iav < nunits:
                    u = units[iav]
                    emit_AV(iav)
                    if u[0] == "diag" and u[3] == 3:  # head-chunk complete
                        recips[(u[1], u[2])] = emit_recip(u[1], u[2])
                inorm = i - LAG - 2
                if 0 <= inorm < nunits:
                    u = units[inorm]
                    if u[0] == "diag" and u[3] == 3:
                        emit_norm(u[1], u[2], recips.pop((u[1], u[2])))
                for f in fill_at.get(i, ()):
                    f()
            # drain: run all remaining matmuls ahead of their epilogues so the
            # PE never waits on a DVE/DMA round-trip; epi(4)/epi(5) go first
            # to release the pp ring slots that m-tiles 6/7 need
            for m in range(6):
                oproj3_final_mm(m)
            for m in (4, 5):
                oproj_epilogue(3, m, o3ps[m])
            pps67 = [oproj_mtile_mm(3, m) for m in (6, 7)]
            for m in range(4):
                oproj_epilogue(3, m, o3ps[m])
            for m, pps in zip((6, 7), pps67):
                oproj_epilogue(3, m, pps)

    nc.compile()
    return nc


def _shards(W_attn, b_attn, W_proj, b_proj):
    """Per-TP-half weight shards (t = 0, 1), packed for the device layout."""
    shards = []
    for t in range(2):
        heads = range(t * HPC, (t + 1) * HPC)
        wqk = np.empty((C, HPC * 128), np.float16)
        bqk = np.empty(HPC * 128, np.float32)
        wv = np.empty((C, HPC * D), np.float16)
        bvv = np.empty(HPC * D, np.float32)
        for j, h in enumerate(heads):
            # m-tile 2g holds q of heads (2g, 2g+1); m-tile 2g+1 their k
            qcol = (j // 2) * 256 + (j % 2) * 64
            kcol = qcol + 128
            wqk[:, qcol : qcol + 64] = W_attn[:, h * D : (h + 1) * D]
            wqk[:, kcol : kcol + 64] = W_attn[:, C + h * D : C + (h + 1) * D]
            bqk[qcol : qcol + 64] = b_attn[h * D : (h + 1) * D]
            bqk[kcol : kcol + 64] = b_attn[C + h * D : C + (h + 1) * D]
            wv[:, j * D : (j + 1) * D] = W_attn[:, 2 * C + h * D : 2 * C + (h + 1) * D]
            bvv[j * D : (j + 1) * D] = b_attn[2 * C + h * D : 2 * C + (h + 1) * D]
        wp = W_proj[t * HPC * D : (t + 1) * HPC * D, :].astype(np.float16)
        bpp = (b_proj if t == 0 else np.zeros_like(b_proj)).astype(np.float32)
        shards.append((wqk, bqk, wv, bvv, np.ascontiguousarray(wp), bpp))
    return shards


def _tri_np():
    kr = np.arange(128)[:, None]
    cc = np.arange(128)[None, :]
    return (kr <= cc).astype(np.float16)


def _in_maps(x, W_attn, b_attn, W_proj, b_proj):
    shards = _shards(W_attn, b_attn, W_proj, b_proj)
    tri = _tri_np()
    in_maps = []
    for b in range(B):
        xTb = np.ascontiguousarray(x[b].T.astype(np.float16))
        for t in range(2):
            wqk, bqk, wv, bvv, wp, bpp = shards[t]
            in_maps.append(
                {
                    "xT": xTb,
                    "wqk": wqk,
                    "bqk": bqk,
                    "wv": wv,
                    "bv": bvv,
                    "wp": wp,
                    "bp": bpp,
                    "tri": tri,
                }
            )
    return in_maps


def _gather(results):
    out = np.empty((B, T, C), np.float32)
    for b in range(B):
        acc = results[2 * b]["outT"] + results[2 * b + 1]["outT"]
        out[b] = acc.T
    return out


def kernel(x, W_attn, b_attn, W_proj, b_proj):
    x = np.asarray(x, np.float32)
    W_attn = np.asarray(W_attn, np.float32)
    b_attn = np.asarray(b_attn, np.float32)
    W_proj = np.asarray(W_proj, np.float32)
    b_proj = np.asarray(b_proj, np.float32)

    if "nc" not in _CACHE:
        _CACHE["nc"] = _build()
    nc = _CACHE["nc"]

    in_maps = _in_maps(x, W_attn, b_attn, W_proj, b_proj)
    # First execution after process start can see device state perturbed by
    # the PJRT/axon client init (stale semaphore values on core 0), which
    # lets an engine wait pass early and corrupts the last q-chunk.  The
    # kernel's own end-of-run drain resets all semaphores, so run the NEFF
    # once as a warmup and return the second execution's results.
    run_bass_kernel_spmd(nc, in_maps, core_ids=list(range(8)))
    res = run_bass_kernel_spmd(nc, in_maps, core_ids=list(range(8)))
    return _gather(res.results)


# revision 44
# speedup vs baseline: 1.0512x; 1.0512x over previous
"""Causal self-attention (B=4, T=2048, C=1024, H=16) on 8 Trainium2 cores.

Sharding: data-parallel over batch (4 groups) x tensor-parallel over heads
(2-way). Core c = 2*b + t handles batch b, heads [t*8, t*8+8).

v3 design notes (flat ACT:PE ratio schedule):
  v2 ran qc-major phases; its final phase (qc=3 attention) had per-window
  scalar-engine exp load ~= PE load, so the PE starved behind exp, the DVFS
  controller dropped the PE clock to ~1.2 GHz for the last ~90us, and every
  tail matmul ran ~2x slow.  v3 keeps v2's math and per-unit structure but
  re-orders the whole kernel as two blocks with a uniform exp:matmul mix:

  - block A: per head, pair (0,h) then (1,h)   (10 units/head)
  - block B: per head, pair (2,h) then (3,h)   (18 units/head)
  Projection m-tiles are spread just-in-time as PE fillers so every window
  keeps PE work ~1.2x the exp work: vproj(0..1)+qkproj(0..1) inside A,
  vproj(2..3)+qkproj(2..3)+oproj(0..1) inside B, oproj(2) in the (3,h7)
  window, oproj(3) staged as partials into the drain.

  Engine re-balance: the PSUM->SBUF moves that v2 ran on DVE/ACT (qk bias
  add, v bias add, early oproj epilogues) go to the mostly-idle Pool
  (gpsimd) engine; ACT does exp only; DVE keeps tri-mask, l-copy/recip and
  the softmax normalize; output DMA issues from the SP (sync) queue instead
  of Pool's SWDGE.  S->AV lag raised to 4 units for more exp-latency slack.

Per-core math (all matmuls fp16 in / fp32 psum accumulate), as in v2:
  qkv in transposed layout q^T/k^T[feat,T]; V in [T,feat] with a ones column
  so AV also accumulates the softmax denominator; S^T = K^T.T @ Q^T per
  (head, k-block, 512-col q-chunk), causal blocks only; exp on ACT, fused
  across the unit's 2 PSUM banks; multiplicative f16 triangle mask after exp
  on the diagonal blocks; out^T row-parallel = W_proj_half.T @ y^T (+ b_proj
  on the t=0 core); host sums TP partials.
"""

import os
import sys

import numpy as np

from concourse import mybir, tile, bacc
from concourse import bass_utils
from concourse.bass_utils import run_bass_kernel_spmd


def _ensure_trace_support():
    """Make trace=True / BASS_TRACE runs survive on images whose antenv lacks
    axon_hooks and where artifact upload has no credentials. Both shims are
    no-ops on the untraced path."""
    try:
        import antenv.axon_hooks  # noqa: F401
    except ImportError:
        import contextlib
        import ctypes
        import types

        mod = types.ModuleType("antenv.axon_hooks")
        state = {"hook": None, "tried": False}

        def set_axon_ntff_profile_hook(hook):
            state["hook"] = hook

        def _via_ctypes(so_path):
            lib = ctypes.CDLL(so_path)
            if not hasattr(lib, "axon_start_nrt_profile"):
                return None
            lib.axon_start_nrt_profile.argtypes = [
                ctypes.POINTER(ctypes.c_int64),
                ctypes.c_size_t,
            ]
            lib.axon_start_nrt_profile.restype = ctypes.c_int64
            lib.axon_stop_nrt_profile.argtypes = [ctypes.c_char_p]
            lib.axon_stop_nrt_profile.restype = ctypes.c_int64

            @contextlib.contextmanager
            def _hook(output_dir, device_ids):
                import jax

                jax.devices()
                if device_ids:
                    ids = (ctypes.c_int64 * len(device_ids))(*device_ids)
                    rc = lib.axon_start_nrt_profile(ids, len(device_ids))
                else:
                    rc = lib.axon_start_nrt_profile(None, 0)
                if rc != 0:
                    raise RuntimeError(f"axon_start_nrt_profile rc={rc}")
                try:
                    yield
                finally:
                    lib.axon_stop_nrt_profile(str(output_dir).encode())

            return _hook

        def get_axon_ntff_profile_hook():
            if state["hook"] is None and not state["tried"]:
                state["tried"] = True
                so = os.environ.get("AXON_PJRT_SO", "/opt/axon/libaxon_pjrt.so")
                if os.path.exists(so):
                    try:
                        state["hook"] = _via_ctypes(so)
                    except OSError:
                        pass
            return state["hook"]

        mod.set_axon_ntff_profile_hook = set_axon_ntff_profile_hook
        mod.get_axon_ntff_profile_hook = get_axon_ntff_profile_hook
        sys.modules["antenv.axon_hooks"] = mod

    orig_upload = bass_utils.upload_artifacts
    if not getattr(orig_upload, "_safe_wrapped", False):
        def _safe_upload(tmpdir):
            try:
                return orig_upload(tmpdir)
            except Exception:
                return "local://" + str(tmpdir)

        _safe_upload._safe_wrapped = True
        bass_utils.upload_artifacts = _safe_upload


_ensure_trace_support()

F16 = mybir.dt.float16
F32 = mybir.dt.float32
EXPF = mybir.ActivationFunctionType.Exp
IDF = mybir.ActivationFunctionType.Identity

B, T, C, H, D = 4, 2048, 1024, 16, 64
HPC = 8          # heads per core
QC = 512         # q-chunk width
NT = T // 128    # 16 T-tiles of 128
NQC = T // QC    # 4 q-chunks
NKC = C // 128   # 8 contraction tiles for the input projections
NKP = (HPC * D) // 128  # 4 contraction tiles for the output projection
LAG = int(os.environ.get("K_LAG", "4"))  # units between an S group and its AV

_CACHE = {}


def _build():
    nc = bacc.Bacc("TRN2", target_bir_lowering=False, debug=False)

    xT = nc.dram_tensor("xT", [C, T], F16, kind="ExternalInput")
    wqk = nc.dram_tensor("wqk", [C, HPC * 128], F16, kind="ExternalInput")
    bqk = nc.dram_tensor("bqk", [HPC * 128], F32, kind="ExternalInput")
    wv = nc.dram_tensor("wv", [C, HPC * D], F16, kind="ExternalInput")
    bv = nc.dram_tensor("bv", [HPC * D], F32, kind="ExternalInput")
    wp = nc.dram_tensor("wp", [HPC * D, C], F16, kind="ExternalInput")
    bp = nc.dram_tensor("bp", [C], F32, kind="ExternalInput")
    tri = nc.dram_tensor("tri", [128, 128], F16, kind="ExternalInput")
    outT = nc.dram_tensor("outT", [C, T], F32, kind="ExternalOutput")

    with tile.TileContext(nc) as tc:
        with (
            tc.tile_pool(name="wts", bufs=1) as wpool,
            tc.tile_pool(name="qk", bufs=1) as qkpool,
            tc.tile_pool(name="vy", bufs=1) as vypool,
            tc.tile_pool(name="xc", bufs=4) as xpool,
            tc.tile_pool(name="pt", bufs=LAG + 2) as ptpool,
            tc.tile_pool(name="st", bufs=3) as stpool,
            tc.tile_pool(name="sm", bufs=3) as smallpool,
            tc.tile_pool(name="sg", bufs=2, space="PSUM") as spool,
            tc.tile_pool(name="ob", bufs=2, space="PSUM") as opool,
            tc.tile_pool(name="pp", bufs=2, space="PSUM") as ppool,
        ):
            # ---- persistent activation tiles ----
            # head-pair packing: tile 2g = q^T of heads 2g (rows 0-63) and
            # 2g+1 (rows 64-127); tile 2g+1 = k^T of the same pair.  Head h
            # reads q and k at the same base partition 64*(h%2) (a matmul
            # requirement), and each qkproj PSUM drain is one full-width op.
            qk_t = [qkpool.tile([128, T], F16, tag=f"qk{m}", name=f"qk{m}") for m in range(HPC)]
            v_sb = vypool.tile([128, NT, HPC, D + 1], F16, tag="v")
            nc.vector.memset(v_sb[:, :, :, D : D + 1], 1.0)
            y_t = [vypool.tile([128, T], F16, tag=f"y{g}", name=f"y{g}") for g in range(NKP)]

            # ---- startup DMAs in consumption order: first V m-tile needs
            # xc0 + wv; first qk m-tile needs wqk half + bqk; tri is needed by
            # the very first diag unit so it goes out early ----
            xcs = {}
            def dma_xc(qc):
                xc = xpool.tile([128, NKC, QC], F16, tag="xc")
                src = xT.ap()[:, qc * QC : (qc + 1) * QC].rearrange("(a p) n -> p a n", p=128)
                nc.sync.dma_start(xc[:], src)
                xcs[qc] = xc

            # startup order: xc0 (kk-pair chunks, streamed into qkproj's
            # accumulation), bqk, wqk head-pair-0 quarter -- that 1.5MB is all
            # qkproj(0,0)/(0,1) need, so S units start ~4.5us in; wv/bvb/tri
            # follow for the vproj prologue, then the rest of the weights
            xc0 = xpool.tile([128, NKC, QC], F16, tag="xc")
            xc0_src = xT.ap()[:, 0:QC].rearrange("(a p) n -> p a n", p=128)
            wv_sb = wpool.tile([128, NKC, HPC * D], F16, tag="wv")
            wv_src = wv.ap().rearrange("(a p) m -> p a m", p=128)
            wqk_sb = wpool.tile([128, NKC, HPC * 128], F16, tag="wqk")
            wqk_src = wqk.ap().rearrange("(a p) m -> p a m", p=128)
            for kk2 in range(0, NKC, 2):
                nc.sync.dma_start(xc0[:, kk2 : kk2 + 2, :], xc0_src[:, kk2 : kk2 + 2, :])
            xcs[0] = xc0
            bqk_sb = wpool.tile([128, HPC], F32, tag="bqk")
            nc.sync.dma_start(bqk_sb[:], bqk.ap().rearrange("(m p) -> p m", p=128))
            nc.sync.dma_start(wqk_sb[:, :, 0:256], wqk_src[:, :, 0:256])
            for kk2 in range(0, NKC, 2):
                nc.sync.dma_start(wv_sb[:, kk2 : kk2 + 2, :], wv_src[:, kk2 : kk2 + 2, :])
            bvb = wpool.tile([128, HPC * D], F32, tag="bvb")
            nc.sync.dma_start(
                bvb[:],
                bv.ap().rearrange("(o n) -> o n", o=1).partition_broadcast(128),
            )
            # two copies of the triangle side by side so one DVE multiply
            # masks both slots of a diag pair's pt tile
            trisb = wpool.tile([128, 2 * 128], F16, tag="tri")
            nc.sync.dma_start(trisb[:, 0:128], tri.ap())
            nc.sync.dma_start(trisb[:, 128:256], tri.ap())
            for quarter in range(1, 4):
                s = slice(quarter * 256, (quarter + 1) * 256)
                nc.sync.dma_start(wqk_sb[:, :, s], wqk_src[:, :, s])
            dma_xc(1)
            wp_sb = wpool.tile([128, NKP, C], F16, tag="wp")
            nc.sync.dma_start(wp_sb[:], wp.ap().rearrange("(a p) m -> p a m", p=128))
            bp_sb = wpool.tile([128, C // 128], F32, tag="bp")
            nc.sync.dma_start(bp_sb[:], bp.ap().rearrange("(m p) -> p m", p=128))
            dma_xc(2)
            dma_xc(3)

            # ---- projection m-tiles (used as attention fillers) ----
            def vproj_mtile(qc, tt):
                ps = ppool.tile([128, QC], F32, tag="pp", name="psv")
                for kk in range(NKC):
                    nc.tensor.matmul(
                        ps[:],
                        xcs[qc][:, kk, tt * 128 : (tt + 1) * 128],
                        wv_sb[:, kk, :],
                        start=(kk == 0),
                        stop=(kk == NKC - 1),
                    )
                nc.vector.tensor_add(
                    v_sb[:, qc * 4 + tt, :, 0:D],
                    ps[:].rearrange("p (h d) -> p h d", d=D),
                    bvb[:].rearrange("p (h d) -> p h d", d=D),
                )

            def qkproj_mtile(qc, m):
                q0 = qc * QC
                ps = ppool.tile([128, QC], F32, tag="pp", name="psqk")
                for kk in range(NKC):
                    nc.tensor.matmul(
                        ps[:],
                        wqk_sb[:, kk, m * 128 : (m + 1) * 128],
                        xcs[qc][:, kk, :],
                        start=(kk == 0),
                        stop=(kk == NKC - 1),
                    )
                nc.vector.tensor_scalar_add(
                    qk_t[m][:, q0 : q0 + QC], ps[:], bqk_sb[:, m : m + 1]
                )

            def oproj_epilogue(qc, m, pps):
                q0 = qc * QC
                st = stpool.tile([128, QC], F32, tag="st", name="st")
                # chunk-3 epilogues drain after the last exp: split them
                # across ACT and DVE so the final PSUM drain runs in parallel
                if qc == 3 and m % 2 == 0:
                    nc.scalar.activation(st[:], pps[:], IDF, bias=bp_sb[:, m : m + 1])
                else:
                    nc.vector.tensor_scalar_add(st[:], pps[:], bp_sb[:, m : m + 1])
                nc.gpsimd.dma_start(
                    outT.ap()[m * 128 : (m + 1) * 128, q0 : q0 + QC], st[:]
                )
                return st

            def oproj_mtile_mm(qc, m):
                q0 = qc * QC
                pps = ppool.tile([128, QC], F32, tag="pp", name="pso")
                for kk in range(NKP):
                    nc.tensor.matmul(
                        pps[:],
                        wp_sb[:, kk, m * 128 : (m + 1) * 128],
                        y_t[kk][:, q0 : q0 + QC],
                        start=(kk == 0),
                        stop=(kk == NKP - 1),
                    )
                return pps

            def oproj_mtile(qc, m):
                oproj_epilogue(qc, m, oproj_mtile_mm(qc, m))

            # chunk-3 output projection, split so only the kk=3 matmuls (which
            # need the very last normalizes) remain after the unit stream
            # drains. m 0-3 borrow freed S-group banks; m 4,5 use the proj
            # pool (allocated after all other pp users, released by finals).
            o3ps = {}

            def oproj3_partial(m):
                q0 = 3 * QC
                if m in (0, 1):
                    if "A" not in o3ps:
                        o3ps["A"] = spool.tile([128, 2, QC], F32, tag="sg", name="o3A")
                    pps = o3ps["A"][:, m % 2, :]
                elif m in (2, 3):
                    if "B" not in o3ps:
                        o3ps["B"] = spool.tile([128, 2, QC], F32, tag="sg", name="o3B")
                    pps = o3ps["B"][:, m % 2, :]
                else:
                    pps = ppool.tile([128, QC], F32, tag="pp", name="pso3")[:]
                o3ps[m] = pps
                for kk in range(3):
                    nc.tensor.matmul(
                        pps,
                        wp_sb[:, kk, m * 128 : (m + 1) * 128],
                        y_t[kk][:, q0 : q0 + QC],
                        start=(kk == 0),
                        stop=False,
                    )

            def oproj3_final_mm(m):
                q0 = 3 * QC
                nc.tensor.matmul(
                    o3ps[m],
                    wp_sb[:, 3, m * 128 : (m + 1) * 128],
                    y_t[3][:, q0 : q0 + QC],
                    start=False,
                    stop=True,
                )

            # ---- attention unit stream ----
            # unit = ("full", qc, h, g) -> k-blocks 2g, 2g+1 (S pair + fused exp)
            #      | ("diag", qc, h, o) -> k-block 4qc+o, partial width + tri mask
            # Block A: per head, pair (0,h) then (1,h).  Block B: per head,
            # pair (2,h) then (3,h).  This keeps the exp:matmul ratio flat so
            # the scalar engine never backs the PE up (v2's tail problem).
            units = []
            for h in range(HPC):
                units += [("diag", 0, h, o) for o in range(4)]
                units += [("full", 1, h, g) for g in range(2)]
                units += [("diag", 1, h, o) for o in range(4)]
            baseB = len(units)  # 80
            for h in range(HPC):
                units += [("full", 2, h, g) for g in range(4)]
                units += [("diag", 2, h, o) for o in range(4)]
                units += [("full", 3, h, g) for g in range(6)]
                units += [("diag", 3, h, o) for o in range(4)]
            nunits = len(units)  # 224

            # fillers, just-in-time: position p emits after unit p's S (and
            # unit p-LAG's AV), so a filler feeding unit j sits at p <= j-1
            fill_at = {}

            def fill(pos, fn):
                fill_at.setdefault(pos, []).append(fn)

            # block A: vproj(0) under head 0's first units, vproj(1) under its
            # (1,h0) units; each head-pair g's four qkproj m-tiles (Q/K for
            # chunks 0 and 1) spread over the two preceding head windows
            fill(0, lambda: vproj_mtile(0, 1))
            fill(1, lambda: vproj_mtile(0, 2))
            fill(1, lambda: qkproj_mtile(1, 0))
            fill(2, lambda: vproj_mtile(0, 3))
            fill(2, lambda: qkproj_mtile(1, 1))
            for tt in range(4):
                fill(5 + tt, lambda tt=tt: vproj_mtile(1, tt))
            for g in range(1, 4):
                fill(20 * g - 15, lambda m=2 * g: qkproj_mtile(0, m))
                fill(20 * g - 9, lambda m=2 * g + 1: qkproj_mtile(0, m))
                fill(20 * g - 3, lambda m=2 * g: qkproj_mtile(1, m))
                fill(20 * g + 1, lambda m=2 * g + 1: qkproj_mtile(1, m))
            fill(63, lambda: qkproj_mtile(2, 0))
            fill(68, lambda: qkproj_mtile(2, 1))
            # block B: vproj(2)/vproj(3) under head 0's windows, qkproj(3,*)
            # early in each pair, qkproj(2,*) for the next pair late in it,
            # oproj(0)/oproj(1) one m-tile per head window
            for tt in range(4):
                fill(baseB + tt, lambda tt=tt: vproj_mtile(2, tt))
                fill(baseB + 9 + tt, lambda tt=tt: vproj_mtile(3, tt))
            for g in range(4):
                b0, b1 = baseB + 36 * g, baseB + 36 * g + 18
                fill(b0 + 3, lambda m=2 * g: qkproj_mtile(3, m))
                fill(b1 + 1, lambda m=2 * g + 1: qkproj_mtile(3, m))
                if g < 3:
                    fill(b0 + 15, lambda m=2 * g + 2: qkproj_mtile(2, m))
                    fill(b1 + 15, lambda m=2 * g + 3: qkproj_mtile(2, m))
            for h in range(HPC):
                base = baseB + 18 * h
                fill(base + 4, lambda h=h: oproj_mtile(0, h))
                fill(base + 13 if h < 7 else baseB + 18 * 6 + 15,
                     lambda h=h: oproj_mtile(1, h))
            # oproj(2) lands in the (3,h7) window right after (2,h7) is
            # normalized -- exactly where the unit stream runs out of cheap
            # filler; oproj(3) partials go into the drain (the spool-backed
            # ones must follow the last S unit or the pool ring deadlocks)
            for m in range(HPC):
                fill(nunits - 5 + m // 2, lambda m=m: oproj_mtile(2, m))
            fill(nunits + 0, lambda: oproj3_partial(4))
            fill(nunits + 1, lambda: oproj3_partial(5))
            fill(nunits + 2, lambda: oproj3_partial(0))
            fill(nunits + 2, lambda: oproj3_partial(1))
            fill(nunits + 3, lambda: oproj3_partial(2))
            fill(nunits + 3, lambda: oproj3_partial(3))

            o_tiles = {}   # (qc, h) -> O psum tile
            dsg = {}       # (qc, h) -> sgrp tile shared by diag pairs
            pts = {}       # unit index -> pt tile

            def emit_S(i):
                kind, qc, h, a = units[i]
                q0 = qc * QC
                if kind == "full":
                    sg = spool.tile([128, 2, QC], F32, tag="sg", name="sg")
                    qt, kt = qk_t[2 * (h // 2)], qk_t[2 * (h // 2) + 1]
                    b0 = 64 * (h % 2)
                    for b in (0, 1):
                        j = 2 * a + b
                        nc.tensor.matmul(
                            sg[:, b, :],
                            kt[b0 : b0 + 64, j * 128 : (j + 1) * 128],
                            qt[b0 : b0 + 64, q0 : q0 + QC],
                            start=True,
                            stop=True,
                        )
                    pt = ptpool.tile([128, 2, QC], F16, tag="pt2", name="pt2")
                    nc.scalar.activation(
                        pt[:].rearrange("p a n -> p (a n)"),
                        sg[:].rearrange("p a n -> p (a n)"),
                        EXPF,
                        scale=0.125,
                    )
                else:
                    o = a
                    if o % 2 == 0:
                        sg = spool.tile([128, 2, QC], F32, tag="sg", name="sgd")
                        dsg[(qc, h)] = sg
                    else:
                        sg = dsg[(qc, h)]
                    j = 4 * qc + o
                    c0 = 128 * o
                    w = QC - c0
                    qt, kt = qk_t[2 * (h // 2)], qk_t[2 * (h // 2) + 1]
                    b0 = 64 * (h % 2)
                    nc.tensor.matmul(
                        sg[:, o % 2, 0:w],
                        kt[b0 : b0 + 64, j * 128 : (j + 1) * 128],
                        qt[b0 : b0 + 64, q0 + c0 : q0 + QC],
                        start=True,
                        stop=True,
                    )
                    if o == 0:
                        # o=0/1 share one fused exp over the whole sg tile
                        # (slot-1 cols 384: hold stale finite S values whose
                        # exps are never read); fusing saves an ACT op and a
                        # PE wait per diag quad
                        pts[i] = None
                        return
                    if o == 1:
                        pt = ptpool.tile([128, 2, QC], F16, tag="pt2", name="ptd")
                        nc.scalar.activation(
                            pt[:].rearrange("p a n -> p (a n)"),
                            sg[:].rearrange("p a n -> p (a n)"),
                            EXPF,
                            scale=0.125,
                        )
                        nc.vector.tensor_mul(pt[:, 0, 0:128], pt[:, 0, 0:128], trisb[:, 0:128])
                        nc.vector.tensor_mul(pt[:, 1, 0:128], pt[:, 1, 0:128], trisb[:, 0:128])
                        pts[i - 1] = (pt, 0)
                        pts[i] = (pt, 1)
                        return
                    pt = ptpool.tile([128, QC], F16, tag="pt1", name="pt1")
                    nc.scalar.activation(
                        pt[:, 0:w], sg[:, o % 2, 0:w], EXPF, scale=0.125
                    )
                    nc.vector.tensor_mul(pt[:, 0:128], pt[:, 0:128], trisb[:, 0:128])
                pts[i] = pt

            def emit_AV(i):
                kind, qc, h, a = units[i]
                n_k = 4 * qc + 4
                if (qc, h) not in o_tiles:
                    o_tiles[(qc, h)] = opool.tile([128, QC], F32, tag="ob", name="ob")
                ot = o_tiles[(qc, h)]
                pt = pts.pop(i)
                if kind == "full":
                    for b in (0, 1):
                        j = 2 * a + b
                        nc.tensor.matmul(
                            ot[0:65, :],
                            v_sb[:, j, h, :],
                            pt[:, b, :],
                            start=(j == 0),
                            stop=(j == n_k - 1),
                        )
                else:
                    o = a
                    j = 4 * qc + o
                    c0 = 128 * o
                    src = pt[0][:, pt[1], 0 : QC - c0] if isinstance(pt, tuple) else pt[:, 0 : QC - c0]
                    nc.tensor.matmul(
                        ot[0:65, c0:QC],
                        v_sb[:, j, h, :],
                        src,
                        start=(j == 0),
                        stop=(j == n_k - 1),
                    )

            def emit_recip(qc, h):
                # copy l out of PSUM first: the approx-fast custom op's
                # BITWISE_NOT seed reads garbage through the PSUM port
                lrow = smallpool.tile([1, QC], F32, tag="lr", name="lr")
                nc.vector.tensor_copy(lrow[:], o_tiles[(qc, h)][64:65, :])
                r = smallpool.tile([1, QC], F32, tag="r", name="r")
                nc.vector.reciprocal_approx_fast(r[:], lrow[:])
                return r

            def emit_norm(qc, h, r):
                rb = smallpool.tile([64, QC], F32, tag="rb", name="rb")
                nc.gpsimd.partition_broadcast(rb[:], r[:])
                r0 = (h % 2) * 64
                nc.vector.tensor_mul(
                    y_t[h // 2][r0 : r0 + 64, qc * QC : (qc + 1) * QC],
                    o_tiles.pop((qc, h))[0:64, :],
                    rb[:],
                )

            # ---- prologue + pipeline loop ----
            qkproj_mtile(0, 0)
            qkproj_mtile(0, 1)
            vproj_mtile(0, 0)

            recips = {}
            for i in range(nunits + LAG + 2):
                if i < nunits:
                    emit_S(i)
                iav = i - LAG
                if 0 <= iav < nunits:
                    u = units[iav]
                    emit_AV(iav)
                    if u[0] == "diag" and u[3] == 3:  # head-chunk complete
                        recips[(u[1], u[2])] = emit_recip(u[1], u[2])
                inorm = i - LAG - 2
                if 0 <= inorm < nunits:
                    u = units[inorm]
                    if u[0] == "diag" and u[3] == 3:
                        emit_norm(u[1], u[2], recips.pop((u[1], u[2])))
                for f in fill_at.get(i, ()):
                    f()
            # drain: run all remaining matmuls ahead of their epilogues so the
            # PE never waits on a DVE/DMA round-trip; epi(4)/epi(5) go first
            # to release the pp ring slots that m-tiles 6/7 need
            for m in range(6):
                oproj3_final_mm(m)
            for m in (4, 5):
                oproj_epilogue(3, m, o3ps[m])
            pps67 = [oproj_mtile_mm(3, m) for m in (6, 7)]
            for m in range(4):
                oproj_epilogue(3, m, o3ps[m])
            for m, pps in zip((6, 7), pps67):
                oproj_epilogue(3, m, pps)

    nc.compile()
    return nc


def _shards(W_attn, b_attn, W_proj, b_proj):
    """Per-TP-half weight shards (t = 0, 1), packed for the device layout."""
    shards = []
    for t in range(2):
        heads = range(t * HPC, (t + 1) * HPC)
        wqk = np.empty((C, HPC * 128), np.float16)
        bqk = np.empty(HPC * 128, np.float32)
        wv = np.empty((C, HPC * D), np.float16)
        bvv = np.empty(HPC * D, np.float32)
        for j, h in enumerate(heads):
            # m-tile 2g holds q of heads (2g, 2g+1); m-tile 2g+1 their k
            qcol = (j // 2) * 256 + (j % 2) * 64
            kcol = qcol + 128
            wqk[:, qcol : qcol + 64] = W_attn[:, h * D : (h + 1) * D]
            wqk[:, kcol : kcol + 64] = W_attn[:, C + h * D : C + (h + 1) * D]
            bqk[qcol : qcol + 64] = b_attn[h * D : (h + 1) * D]
            bqk[kcol : kcol + 64] = b_attn[C + h * D : C + (h + 1) * D]
            wv[:, j * D : (j + 1) * D] = W_attn[:, 2 * C + h * D : 2 * C + (h + 1) * D]
            bvv[j * D : (j + 1) * D] = b_attn[2 * C + h * D : 2 * C + (h + 1) * D]
        wp = W_proj[t * HPC * D : (t + 1) * HPC * D, :].astype(np.float16)
        bpp = (b_proj if t == 0 else np.zeros_like(b_proj)).astype(np.float32)
        shards.append((wqk, bqk, wv, bvv, np.ascontiguousarray(wp), bpp))
    return shards


def _tri_np():
    kr = np.arange(128)[:, None]
    cc = np.arange(128)[None, :]
    return (kr <= cc).astype(np.float16)


def _in_maps(x, W_attn, b_attn, W_proj, b_proj):
    shards = _shards(W_attn, b_attn, W_proj, b_proj)
    tri = _tri_np()
    in_maps = []
    for b in range(B):
        xTb = np.ascontiguousarray(x[b].T.astype(np.float16))
        for t in range(2):
            wqk, bqk, wv, bvv, wp, bpp = shards[t]
            in_maps.append(
                {
                    "xT": xTb,
                    "wqk": wqk,
                    "bqk": bqk,
                    "wv": wv,
                    "bv": bvv,
                    "wp": wp,
                    "bp": bpp,
                    "tri": tri,
                }
            )
    return in_maps


def _gather(results):
    out = np.empty((B, T, C), np.float32)
    for b in range(B):
        acc = results[2 * b]["outT"] + results[2 * b + 1]["outT"]
        out[b] = acc.T
    return out


def kernel(x, W_attn, b_attn, W_proj, b_proj):
    x = np.asarray(x, np.float32)
    W_attn = np.asarray(W_attn, np.float32)
    b_attn = np.asarray(b_attn, np.float32)
    W_proj = np.asarray(W_proj, np.float32)
    b_proj = np.asarray(b_proj, np.float32)

    if "nc" not in _CACHE:
        _CACHE["nc"] = _build()
    nc = _CACHE["nc"]

    in_maps = _in_maps(x, W_attn, b_attn, W_proj, b_proj)
    # First execution after process start can see device state perturbed by
    # the PJRT/axon client init (stale semaphore values on core 0), which
    # lets an engine wait pass early and corrupts the last q-chunk.  The
    # kernel's own end-of-run drain resets all semaphores, so run the NEFF
    # once as a warmup and return the second execution's results.
    run_bass_kernel_spmd(nc, in_maps, core_ids=list(range(8)))
    res = run_bass_kernel_spmd(nc, in_maps, core_ids=list(range(8)))
    return _gather(res.results)
